# revision 18
# baseline (speedup 1.0000x reference)
"""Cross-attention (1x1-conv QKV + softmax attention + residual) on 8 TRN2 cores.

Sharding: batch (4) x query-half (2) -> 8 shards, one per core. Each core
computes attention for 2048 query tokens of one batch against all 4096
key tokens of that batch, entirely in channel-major [C, N] layout.

Key algebraic fusion: softmax is invariant to per-query shifts, so the
k-bias bk cancels exactly, and the q/k projections collapse into one:
  S^T = k^T q = (Wk x2)^T (Wq x1 + bq) = x2^T qt,
  qt = (Wk^T Wq) x1 + Wk^T bq = Wc x1 + bc   (Wc, bc precomputed on host)
so no K projection runs on device; x2 feeds the S matmuls directly.

Per core:
  qt = Wc x1 + bc                     [C, 512] x4  (bf16)
  v  = x2^T @ wv^T (token-major)      [4096, C]    (fp8 pairs, bias folded later)
  S^T tiles = x2_chunk^T @ qt         [128j, 512i] (PSUM f32)
  P = exp(S^T * 1/sqrt(C))            (ScalarE -> fp8; |S*scale| ~ 2, exp safe)
  O  += v_chunk^T @ P_chunk           [C, 512i]    (PSUM f32, DoubleRow fp8)
  den += ones^T @ P_chunk             [1, 512i]    (DoubleRow fp8)
  out = O * (1/den) + (x1 + bv)       (reciprocal on DVE, broadcast via PE,
                                       v-bias exact after softmax normalization)

The Activation engine (exp: 64 instructions of [128,1024] @ ~1.02us) is the
hard bottleneck (~65us/iter measured in isolation); everything else is
scheduled to keep it saturated.
"""

import os
import sys

import numpy as np

os.environ.setdefault("JAX_COMPILATION_CACHE_DIR", "/tmp/jaxcache")


def _ensure_concourse():
    try:
        import concourse  # noqa: F401
        return
    except ImportError:
        pass
    for p in ("/opt/trn_rl_repo", os.path.expanduser("~/.axon_site/_ro/trn_rl_repo")):
        if os.path.isdir(p):
            sys.path.insert(0, p)
            try:
                import concourse  # noqa: F401
                return
            except ImportError:
                sys.path.remove(p)
    raise ImportError("concourse (bass) not importable")


_ensure_concourse()

import concourse.bass as bass  # noqa: E402
import concourse.mybir as mybir  # noqa: E402
import concourse.tile as tile  # noqa: E402
from concourse import bacc  # noqa: E402
from concourse.bass_utils import run_bass_kernel_spmd  # noqa: E402

F32 = mybir.dt.float32
BF16 = mybir.dt.bfloat16
FP8 = mybir.dt.float8e4

C = 128          # channels / hidden dim
B = 4            # batch
N = 4096         # tokens per batch (64*64)
NQ = 2048        # query tokens per core (half batch)
N_CORES = 8
NJT = N // 128   # 32 key chunks of 128
N_IB = NQ // 512  # 4 query blocks of 512
NG = NJT // 2    # 16 key-chunk pairs per query block
LAG = 3          # PV trails exp by this many groups
SCALE = float(1.0 / np.sqrt(C))
UNROLL = 4       # bodies per hardware-loop iteration (amortizes the
                 # all-engine barrier For_i inserts between iterations)


def build_nc(repeats=1):
    nc = bacc.Bacc("TRN2", target_bir_lowering=False, debug=False,
                   num_devices=N_CORES)

    x1bf = nc.declare_dram_parameter("x1bf", [C, NQ], BF16, isOutput=False)
    x2f = nc.declare_dram_parameter("x2f", [C, N], BF16, isOutput=False)
    wcT = nc.declare_dram_parameter("wcT", [C, C], BF16, isOutput=False)
    wvT = nc.declare_dram_parameter("wvT", [C, C], BF16, isOutput=False)
    bcv = nc.declare_dram_parameter("bcv", [C, 2], F32, isOutput=False)
    out = nc.declare_dram_parameter("out", [C, NQ], F32, isOutput=True)

    with tile.TileContext(nc) as tc:
        with (
            tc.tile_pool(name="const", bufs=1) as cpool,
            tc.tile_pool(name="persist", bufs=1) as ppool,
            tc.tile_pool(name="work", bufs=2) as wpool,
            tc.tile_pool(name="ps_s", bufs=2, space="PSUM") as ps_s,
            tc.tile_pool(name="ps_od", bufs=2, space="PSUM") as ps_od,
            tc.tile_pool(name="ps_aux", bufs=2, space="PSUM") as ps_aux,
        ):
            pools = (cpool, ppool, wpool, ps_s, ps_od, ps_aux)
            if repeats == 1:
                _build_body(nc, pools, x1bf, x2f, wcT, wvT, bcv, out)
            else:
                hints = (mybir.EngineType.PE, mybir.EngineType.Activation,
                         mybir.EngineType.DVE, mybir.EngineType.SP,
                         mybir.EngineType.Pool)
                u = UNROLL if repeats % UNROLL == 0 else 1
                with tc.For_i(0, repeats // u, 1, hint_engines=hints):
                    for _ in range(u):
                        _build_body(nc, pools, x1bf, x2f, wcT, wvT,
                                    bcv, out)

    nc.compile()
    return nc


def _build_body(nc, pools, x1bf, x2f, wcT, wvT, bcv, out):
    (cpool, ppool, wpool, ps_s, ps_od, ps_aux) = pools

    # ---- constants -----------------------------------------------------
    wc_b = cpool.tile([C, C], BF16, tag="wcb", name="wcb")
    nc.sync.dma_start(wc_b[:], wcT[:])
    bias_t = cpool.tile([C, 2], F32, tag="bias", name="bias_t")
    b_c = bias_t[:, 0:1]
    b_v = bias_t[:, 1:2]
    # DoubleRow all-ones weights with M=128: the den matmul then writes
    # den broadcast across all 128 partitions, so the epilogue needs no
    # separate PE broadcast of 1/den.
    ones_dr = cpool.tile([C, 2, 128], FP8, tag="ones_dr", name="ones_dr")
    nc.vector.memset(ones_dr[:], 1.0)

    # ---- input DMAs, ordered by need time ------------------------------
    # x1bf(0) + wc gate the first qt projection; x2 chunks gate the S
    # stream; x1h (residual) is needed only at each block epilogue.
    x1b = [None] * N_IB
    x2b = [None] * 8
    x1c = [None] * N_IB

    def dma_x1b(ci):
        xb = ppool.tile([C, 512], BF16, tag=f"x1b{ci}", name=f"x1b{ci}")
        nc.sync.dma_start(xb[:], x1bf[:, ci * 512:(ci + 1) * 512])
        x1b[ci] = xb

    def dma_x2b(ci):
        xb = ppool.tile([C, 512], BF16, tag=f"x2b{ci}", name=f"x2b{ci}")
        nc.sync.dma_start(xb[:], x2f[:, ci * 512:(ci + 1) * 512])
        x2b[ci] = xb

    dma_x1b(0)
    nc.sync.dma_start(bias_t[:], bcv[:])
    wv_b = cpool.tile([C, C], BF16, tag="wvb", name="wvb")
    nc.sync.dma_start(wv_b[:], wvT[:])
    for ci in range(8):
        dma_x2b(ci)
    dma_x1b(1)
    dma_x1b(2)
    dma_x1b(3)

    # ---- helpers -------------------------------------------------------
    qt = [None] * N_IB

    def emit_qt(ib):
        q_ps = ps_aux.tile([C, 512], F32, tag="aux", name=f"qps{ib}")
        nc.tensor.matmul(q_ps[:], wc_b[:], x1b[ib][:], start=True, stop=True)
        q = ppool.tile([C, 512], BF16, tag=f"qt{ib}", name=f"qt{ib}")
        nc.vector.tensor_scalar_add(q[:], q_ps[:], b_c[:])
        qt[ib] = q

    vquad = [None] * 8

    def emit_vq(ci):
        # v rows for key chunks 4ci..4ci+3: four projections into one PSUM
        # bank, one DVE copy out to an fp8 [128, 4, C] tile whose [.,2h:2h+2,.]
        # slices are the DoubleRow lhsT pairs.
        v_ps = ps_aux.tile([128, 512], F32, tag="aux", name=f"vps{ci}")
        for h in range(4):
            jt = 4 * ci + h
            lhs = x2b[jt // 4][:, (jt % 4) * 128:(jt % 4) * 128 + 128]
            nc.tensor.matmul(v_ps[:, h * 128:(h + 1) * 128], lhs, wv_b[:],
                             start=True, stop=True)
        vq = ppool.tile([128, 4, C], FP8, tag=f"vq{ci}", name=f"vq{ci}")
        nc.vector.tensor_copy(vq.rearrange("p f c -> p (f c)"), v_ps[:])
        vquad[ci] = vq

    def emit_xc_bias(ci):
        # residual base: x1 + bv (bf16 x1 is within tolerance; bv exact
        # after softmax normalization)
        xc = ppool.tile([C, 512], F32, tag=f"x1c{ci}", name=f"x1c{ci}")
        nc.vector.tensor_scalar_add(xc[:], x1b[ci][:], b_v[:])
        x1c[ci] = xc

    def emit_s(ib, g):
        jt0, jt1 = 2 * g, 2 * g + 1
        s_ps = ps_s.tile([128, 1024], F32, tag="s", name=f"sps{ib}_{g}")
        k0 = x2b[jt0 // 4][:, (jt0 % 4) * 128:(jt0 % 4) * 128 + 128]
        k1 = x2b[jt1 // 4][:, (jt1 % 4) * 128:(jt1 % 4) * 128 + 128]
        nc.tensor.matmul(s_ps[:, 0:512], k0, qt[ib][:], start=True, stop=True)
        nc.tensor.matmul(s_ps[:, 512:1024], k1, qt[ib][:],
                         start=True, stop=True)
        e = wpool.tile([128, 1024], FP8, tag="e", bufs=8, name=f"e{ib}_{g}")
        nc.scalar.activation(e[:], s_ps[:], mybir.ActivationFunctionType.Exp,
                             bias=0.0, scale=SCALE)
        return e

    def emit_pv(g, e, o_ps, den_ps):
        first = g == 0
        last = g == NG - 1
        epair = e.rearrange("p (two n) -> p two n", two=2)
        vp = vquad[g // 2][:, 2 * (g % 2):2 * (g % 2) + 2, :]
        nc.tensor.matmul(o_ps[:], vp, epair, start=first, stop=last,
                         perf_mode=mybir.MatmulPerfMode.DoubleRow)
        nc.tensor.matmul(den_ps[:], ones_dr[:], epair,
                         start=first, stop=last,
                         perf_mode=mybir.MatmulPerfMode.DoubleRow)

    def emit_epilogue(ib, o_ps, den_ps):
        # den_ps is already broadcast across partitions (ones_dr M=128):
        # 1/den -> SBUF, multiply into o, add residual, store.
        recip = wpool.tile([C, 512], F32, tag="recip", bufs=2,
                           name=f"recip{ib}")
        nc.vector.reciprocal(recip[:], den_ps[:])
        ob = wpool.tile([C, 512], F32, tag="ob", bufs=2, name=f"ob{ib}")
        nc.vector.tensor_mul(ob[:], o_ps[:], recip[:])
        nc.vector.tensor_add(ob[:], ob[:], x1c[ib][:])
        nc.sync.dma_start(out[:, ib * 512:(ib + 1) * 512], ob[:])

    # ---- main attention stream -----------------------------------------
    # Block ib: 16 groups of (2 S-matmuls -> exp), with PV/den trailing by
    # LAG groups so ScalarE exp latency hides behind PE work. V projections
    # are emitted during block 0 just ahead of first use; qt(ib+1) is
    # emitted early inside block ib.
    emit_qt(0)
    for ib in range(N_IB):
        o_ps = ps_od.tile([C, 512], F32, tag="od", name=f"ops{ib}")
        den_ps = ps_od.tile([C, 512], F32, tag="od", name=f"den{ib}")
        e_q = []
        for g in range(NG):
            if ib == 0 and g < 8:
                # keep v quads comfortably ahead of the PV stream
                # (quad ci covers pairs 2ci, 2ci+1, first used at g=2ci+LAG)
                emit_vq(g)
            e_q.append(emit_s(ib, g))
            if ib == 0 and g == 4:
                emit_xc_bias(0)
            if g == 5 and ib + 1 < N_IB:
                emit_qt(ib + 1)
            if g == 7 and ib + 1 < N_IB:
                emit_xc_bias(ib + 1)
            if g >= LAG:
                emit_pv(g - LAG, e_q[g - LAG], o_ps, den_ps)
        # flush trailing PV groups, then normalize + residual + store
        for g in range(NG - LAG, NG):
            emit_pv(g, e_q[g], o_ps, den_ps)
        emit_epilogue(ib, o_ps, den_ps)


_NC_CACHE = None


def _get_nc():
    global _NC_CACHE
    if _NC_CACHE is None:
        _NC_CACHE = build_nc()
    return _NC_CACHE


def make_in_maps(x1, x2, wq, bq, wk, bk, wv, bv):
    x1 = np.asarray(x1, np.float32)
    x2 = np.asarray(x2, np.float32)
    t1 = np.ascontiguousarray(x1.reshape(B, C, N))
    t2 = np.ascontiguousarray(x2.reshape(B, C, N))
    import ml_dtypes
    bf = ml_dtypes.bfloat16
    wq = np.asarray(wq, np.float64)
    wk = np.asarray(wk, np.float64)
    # fused q/k projection: S^T = x2^T (Wc x1 + bc), bk cancels in softmax
    wc = wq.T @ wk                      # lhsT layout [c_in(x1), c_mid]
    bc = wk.T @ np.asarray(bq, np.float64)
    shared = {
        "wcT": np.ascontiguousarray(wc.astype(np.float32).astype(bf)),
        "wvT": np.ascontiguousarray(
            np.asarray(wv, np.float32).T.astype(bf)),
        "bcv": np.ascontiguousarray(np.stack(
            [bc.astype(np.float32), np.asarray(bv, np.float32)], axis=1)),
    }
    in_maps = []
    for core in range(N_CORES):
        b, h = core // 2, core % 2
        in_maps.append({
            "x1bf": np.ascontiguousarray(
                t1[b][:, h * NQ:(h + 1) * NQ]).astype(bf),
            "x2f": t2[b].astype(bf),
            **shared,
        })
    return in_maps


def assemble_out(results):
    out = np.empty((B, C, N), np.float32)
    for core in range(N_CORES):
        b, h = core // 2, core % 2
        out[b][:, h * NQ:(h + 1) * NQ] = results[core]["out"]
    return out.reshape(B, C, 64, 64)


def kernel(x1, x2, wq, bq, wk, bk, wv, bv):
    nc = _get_nc()
    in_maps = make_in_maps(x1, x2, wq, bq, wk, bk, wv, bv)
    res = run_bass_kernel_spmd(nc, in_maps, list(range(N_CORES)))
    return assemble_out(res.results)


# revision 19
# speedup vs baseline: 1.0011x; 1.0011x over previous
"""Cross-attention (1x1-conv QKV + softmax attention + residual) on 8 TRN2 cores.

Sharding: batch (4) x query-half (2) -> 8 shards, one per core. Each core
computes attention for 2048 query tokens of one batch against all 4096
key tokens of that batch, entirely in channel-major [C, N] layout.

Key algebraic fusion: softmax is invariant to per-query shifts, so the
k-bias bk cancels exactly, and the q/k projections collapse into one:
  S^T = k^T q = (Wk x2)^T (Wq x1 + bq) = x2^T qt,
  qt = (Wk^T Wq) x1 + Wk^T bq = Wc x1 + bc   (Wc, bc precomputed on host)
so no K projection runs on device; x2 feeds the S matmuls directly.

Per core:
  qt = Wc x1 + bc                     [C, 512] x4  (bf16)
  v  = x2^T @ wv^T (token-major)      [4096, C]    (fp8 pairs, bias folded later)
  S^T tiles = x2_chunk^T @ qt         [128j, 512i] (PSUM f32)
  P = exp(S^T * 1/sqrt(C))            (ScalarE -> fp8; |S*scale| ~ 2, exp safe)
  O  += v_chunk^T @ P_chunk           [C, 512i]    (PSUM f32, DoubleRow fp8)
  den += ones^T @ P_chunk             [1, 512i]    (DoubleRow fp8)
  out = O * (1/den) + (x1 + bv)       (reciprocal on DVE, broadcast via PE,
                                       v-bias exact after softmax normalization)

The Activation engine (exp: 64 instructions of [128,1024] @ ~1.02us) is the
hard bottleneck (~65us/iter measured in isolation); everything else is
scheduled to keep it saturated.
"""

import os
import sys

import numpy as np

os.environ.setdefault("JAX_COMPILATION_CACHE_DIR", "/tmp/jaxcache")


def _ensure_concourse():
    try:
        import concourse  # noqa: F401
        return
    except ImportError:
        pass
    for p in ("/opt/trn_rl_repo", os.path.expanduser("~/.axon_site/_ro/trn_rl_repo")):
        if os.path.isdir(p):
            sys.path.insert(0, p)
            try:
                import concourse  # noqa: F401
                return
            except ImportError:
                sys.path.remove(p)
    raise ImportError("concourse (bass) not importable")


_ensure_concourse()

import concourse.bass as bass  # noqa: E402
import concourse.mybir as mybir  # noqa: E402
import concourse.tile as tile  # noqa: E402
from concourse import bacc  # noqa: E402
from concourse.bass_utils import run_bass_kernel_spmd  # noqa: E402

F32 = mybir.dt.float32
BF16 = mybir.dt.bfloat16
FP8 = mybir.dt.float8e4

C = 128          # channels / hidden dim
B = 4            # batch
N = 4096         # tokens per batch (64*64)
NQ = 2048        # query tokens per core (half batch)
N_CORES = 8
NJT = N // 128   # 32 key chunks of 128
N_IB = NQ // 512  # 4 query blocks of 512
NG = NJT // 2    # 16 key-chunk pairs per query block
LAG = 3          # PV trails exp by this many groups
SCALE = float(1.0 / np.sqrt(C))
UNROLL = 8       # bodies per hardware-loop iteration (amortizes the
                 # all-engine barrier For_i inserts between iterations)


def build_nc(repeats=1):
    nc = bacc.Bacc("TRN2", target_bir_lowering=False, debug=False,
                   num_devices=N_CORES)

    x1bf = nc.declare_dram_parameter("x1bf", [C, NQ], BF16, isOutput=False)
    x2f = nc.declare_dram_parameter("x2f", [C, N], BF16, isOutput=False)
    wcT = nc.declare_dram_parameter("wcT", [C, C], BF16, isOutput=False)
    wvT = nc.declare_dram_parameter("wvT", [C, C], BF16, isOutput=False)
    bcv = nc.declare_dram_parameter("bcv", [C, 2], F32, isOutput=False)
    out = nc.declare_dram_parameter("out", [C, NQ], F32, isOutput=True)

    with tile.TileContext(nc) as tc:
        with (
            tc.tile_pool(name="const", bufs=1) as cpool,
            tc.tile_pool(name="persist", bufs=1) as ppool,
            tc.tile_pool(name="work", bufs=2) as wpool,
            tc.tile_pool(name="ps_s", bufs=2, space="PSUM") as ps_s,
            tc.tile_pool(name="ps_od", bufs=2, space="PSUM") as ps_od,
            tc.tile_pool(name="ps_aux", bufs=2, space="PSUM") as ps_aux,
        ):
            pools = (cpool, ppool, wpool, ps_s, ps_od, ps_aux)
            if repeats == 1:
                _build_body(nc, pools, x1bf, x2f, wcT, wvT, bcv, out)
            else:
                hints = (mybir.EngineType.PE, mybir.EngineType.Activation,
                         mybir.EngineType.DVE, mybir.EngineType.SP,
                         mybir.EngineType.Pool)
                u = UNROLL if repeats % UNROLL == 0 else 1
                with tc.For_i(0, repeats // u, 1, hint_engines=hints):
                    for _ in range(u):
                        _build_body(nc, pools, x1bf, x2f, wcT, wvT,
                                    bcv, out)

    nc.compile()
    return nc


def _build_body(nc, pools, x1bf, x2f, wcT, wvT, bcv, out):
    (cpool, ppool, wpool, ps_s, ps_od, ps_aux) = pools

    # ---- constants -----------------------------------------------------
    wc_b = cpool.tile([C, C], BF16, tag="wcb", name="wcb")
    nc.sync.dma_start(wc_b[:], wcT[:])
    bias_t = cpool.tile([C, 2], F32, tag="bias", name="bias_t")
    b_c = bias_t[:, 0:1]
    b_v = bias_t[:, 1:2]
    # DoubleRow all-ones weights with M=128: the den matmul then writes
    # den broadcast across all 128 partitions, so the epilogue needs no
    # separate PE broadcast of 1/den.
    ones_dr = cpool.tile([C, 2, 128], FP8, tag="ones_dr", name="ones_dr")
    nc.vector.memset(ones_dr[:], 1.0)

    # ---- input DMAs, ordered by need time ------------------------------
    # x1bf(0) + wc gate the first qt projection; x2 chunks gate the S
    # stream; x1h (residual) is needed only at each block epilogue.
    x1b = [None] * N_IB
    x2b = [None] * 8
    x1c = [None] * N_IB

    def dma_x1b(ci):
        xb = ppool.tile([C, 512], BF16, tag=f"x1b{ci}", name=f"x1b{ci}")
        nc.sync.dma_start(xb[:], x1bf[:, ci * 512:(ci + 1) * 512])
        x1b[ci] = xb

    def dma_x2b(ci):
        xb = ppool.tile([C, 512], BF16, tag=f"x2b{ci}", name=f"x2b{ci}")
        nc.sync.dma_start(xb[:], x2f[:, ci * 512:(ci + 1) * 512])
        x2b[ci] = xb

    dma_x1b(0)
    nc.sync.dma_start(bias_t[:], bcv[:])
    wv_b = cpool.tile([C, C], BF16, tag="wvb", name="wvb")
    nc.sync.dma_start(wv_b[:], wvT[:])
    for ci in range(8):
        dma_x2b(ci)
    dma_x1b(1)
    dma_x1b(2)
    dma_x1b(3)

    # ---- helpers -------------------------------------------------------
    qt = [None] * N_IB

    def emit_qt(ib):
        q_ps = ps_aux.tile([C, 512], F32, tag="aux", name=f"qps{ib}")
        nc.tensor.matmul(q_ps[:], wc_b[:], x1b[ib][:], start=True, stop=True)
        q = ppool.tile([C, 512], BF16, tag=f"qt{ib}", name=f"qt{ib}")
        nc.vector.tensor_scalar_add(q[:], q_ps[:], b_c[:])
        qt[ib] = q

    vquad = [None] * 8

    def emit_vq(ci):
        # v rows for key chunks 4ci..4ci+3: four projections into one PSUM
        # bank, one DVE copy out to an fp8 [128, 4, C] tile whose [.,2h:2h+2,.]
        # slices are the DoubleRow lhsT pairs.
        v_ps = ps_aux.tile([128, 512], F32, tag="aux", name=f"vps{ci}")
        for h in range(4):
            jt = 4 * ci + h
            lhs = x2b[jt // 4][:, (jt % 4) * 128:(jt % 4) * 128 + 128]
            nc.tensor.matmul(v_ps[:, h * 128:(h + 1) * 128], lhs, wv_b[:],
                             start=True, stop=True)
        vq = ppool.tile([128, 4, C], FP8, tag=f"vq{ci}", name=f"vq{ci}")
        nc.vector.tensor_copy(vq.rearrange("p f c -> p (f c)"), v_ps[:])
        vquad[ci] = vq

    def emit_xc_bias(ci):
        # residual base: x1 + bv (bf16 x1 is within tolerance; bv exact
        # after softmax normalization)
        xc = ppool.tile([C, 512], F32, tag=f"x1c{ci}", name=f"x1c{ci}")
        nc.vector.tensor_scalar_add(xc[:], x1b[ci][:], b_v[:])
        x1c[ci] = xc

    def emit_s(ib, g):
        jt0, jt1 = 2 * g, 2 * g + 1
        s_ps = ps_s.tile([128, 1024], F32, tag="s", name=f"sps{ib}_{g}")
        k0 = x2b[jt0 // 4][:, (jt0 % 4) * 128:(jt0 % 4) * 128 + 128]
        k1 = x2b[jt1 // 4][:, (jt1 % 4) * 128:(jt1 % 4) * 128 + 128]
        nc.tensor.matmul(s_ps[:, 0:512], k0, qt[ib][:], start=True, stop=True)
        nc.tensor.matmul(s_ps[:, 512:1024], k1, qt[ib][:],
                         start=True, stop=True)
        e = wpool.tile([128, 1024], FP8, tag="e", bufs=8, name=f"e{ib}_{g}")
        nc.scalar.activation(e[:], s_ps[:], mybir.ActivationFunctionType.Exp,
                             bias=0.0, scale=SCALE)
        return e

    def emit_pv(g, e, o_ps, den_ps):
        first = g == 0
        last = g == NG - 1
        epair = e.rearrange("p (two n) -> p two n", two=2)
        vp = vquad[g // 2][:, 2 * (g % 2):2 * (g % 2) + 2, :]
        nc.tensor.matmul(o_ps[:], vp, epair, start=first, stop=last,
                         perf_mode=mybir.MatmulPerfMode.DoubleRow)
        nc.tensor.matmul(den_ps[:], ones_dr[:], epair,
                         start=first, stop=last,
                         perf_mode=mybir.MatmulPerfMode.DoubleRow)

    def emit_epilogue(ib, o_ps, den_ps):
        # den_ps is already broadcast across partitions (ones_dr M=128):
        # 1/den -> SBUF, multiply into o, add residual, store.
        recip = wpool.tile([C, 512], F32, tag="recip", bufs=2,
                           name=f"recip{ib}")
        nc.vector.reciprocal(recip[:], den_ps[:])
        ob = wpool.tile([C, 512], F32, tag="ob", bufs=2, name=f"ob{ib}")
        nc.vector.tensor_mul(ob[:], o_ps[:], recip[:])
        nc.vector.tensor_add(ob[:], ob[:], x1c[ib][:])
        nc.sync.dma_start(out[:, ib * 512:(ib + 1) * 512], ob[:])

    # ---- main attention stream -----------------------------------------
    # Block ib: 16 groups of (2 S-matmuls -> exp), with PV/den trailing by
    # LAG groups so ScalarE exp latency hides behind PE work. V projections
    # are emitted during block 0 just ahead of first use; qt(ib+1) is
    # emitted early inside block ib.
    emit_qt(0)
    for ib in range(N_IB):
        o_ps = ps_od.tile([C, 512], F32, tag="od", name=f"ops{ib}")
        den_ps = ps_od.tile([C, 512], F32, tag="od", name=f"den{ib}")
        e_q = []
        for g in range(NG):
            if ib == 0 and g < 8:
                # keep v quads comfortably ahead of the PV stream
                # (quad ci covers pairs 2ci, 2ci+1, first used at g=2ci+LAG)
                emit_vq(g)
            e_q.append(emit_s(ib, g))
            if ib == 0 and g == 4:
                emit_xc_bias(0)
            if g == 5 and ib + 1 < N_IB:
                emit_qt(ib + 1)
            if g == 7 and ib + 1 < N_IB:
                emit_xc_bias(ib + 1)
            if g >= LAG:
                emit_pv(g - LAG, e_q[g - LAG], o_ps, den_ps)
        # flush trailing PV groups, then normalize + residual + store
        for g in range(NG - LAG, NG):
            emit_pv(g, e_q[g], o_ps, den_ps)
        emit_epilogue(ib, o_ps, den_ps)


_NC_CACHE = None


def _get_nc():
    global _NC_CACHE
    if _NC_CACHE is None:
        _NC_CACHE = build_nc()
    return _NC_CACHE


def make_in_maps(x1, x2, wq, bq, wk, bk, wv, bv):
    x1 = np.asarray(x1, np.float32)
    x2 = np.asarray(x2, np.float32)
    t1 = np.ascontiguousarray(x1.reshape(B, C, N))
    t2 = np.ascontiguousarray(x2.reshape(B, C, N))
    import ml_dtypes
    bf = ml_dtypes.bfloat16
    wq = np.asarray(wq, np.float64)
    wk = np.asarray(wk, np.float64)
    # fused q/k projection: S^T = x2^T (Wc x1 + bc), bk cancels in softmax
    wc = wq.T @ wk                      # lhsT layout [c_in(x1), c_mid]
    bc = wk.T @ np.asarray(bq, np.float64)
    shared = {
        "wcT": np.ascontiguousarray(wc.astype(np.float32).astype(bf)),
        "wvT": np.ascontiguousarray(
            np.asarray(wv, np.float32).T.astype(bf)),
        "bcv": np.ascontiguousarray(np.stack(
            [bc.astype(np.float32), np.asarray(bv, np.float32)], axis=1)),
    }
    in_maps = []
    for core in range(N_CORES):
        b, h = core // 2, core % 2
        in_maps.append({
            "x1bf": np.ascontiguousarray(
                t1[b][:, h * NQ:(h + 1) * NQ]).astype(bf),
            "x2f": t2[b].astype(bf),
            **shared,
        })
    return in_maps


def assemble_out(results):
    out = np.empty((B, C, N), np.float32)
    for core in range(N_CORES):
        b, h = core // 2, core % 2
        out[b][:, h * NQ:(h + 1) * NQ] = results[core]["out"]
    return out.reshape(B, C, 64, 64)


def kernel(x1, x2, wq, bq, wk, bk, wv, bv):
    nc = _get_nc()
    in_maps = make_in_maps(x1, x2, wq, bq, wk, bk, wv, bv)
    res = run_bass_kernel_spmd(nc, in_maps, list(range(N_CORES)))
    return assemble_out(res.results)


# revision 21
# speedup vs baseline: 1.0103x; 1.0092x over previous
"""Cross-attention (1x1-conv QKV + softmax attention + residual) on 8 TRN2 cores.

Sharding: batch (4) x query-half (2) -> 8 shards, one per core. Each core
computes attention for 2048 query tokens of one batch against all 4096
key tokens of that batch, entirely in channel-major [C, N] layout.

Key algebraic fusion: softmax is invariant to per-query shifts, so the
k-bias bk cancels exactly, and the q/k projections collapse into one:
  S^T = k^T q = (Wk x2)^T (Wq x1 + bq) = x2^T qt,
  qt = (Wk^T Wq) x1 + Wk^T bq = Wc x1 + bc   (Wc, bc precomputed on host)
so no K projection runs on device; x2 feeds the S matmuls directly.

Per core:
  qt = Wc x1 + bc                     [C, 512] x4  (bf16)
  v  = x2^T @ wv^T (token-major)      [4096, C]    (fp8 pairs, bias folded later)
  S^T tiles = x2_chunk^T @ qt         [128j, 512i] (PSUM f32)
  P = exp(S^T * 1/sqrt(C))            (ScalarE -> fp8; |S*scale| ~ 2, exp safe)
  O  += v_chunk^T @ P_chunk           [C, 512i]    (PSUM f32, DoubleRow fp8)
  den += ones^T @ P_chunk             [1, 512i]    (DoubleRow fp8)
  out = O * (1/den) + (x1 + bv)       (reciprocal on DVE, broadcast via PE,
                                       v-bias exact after softmax normalization)

The Activation engine (exp: 64 instructions of [128,1024] @ ~1.02us) is the
hard bottleneck (~65us/iter measured in isolation); everything else is
scheduled to keep it saturated.
"""

import os
import sys

import numpy as np

os.environ.setdefault("JAX_COMPILATION_CACHE_DIR", "/tmp/jaxcache")


def _ensure_concourse():
    try:
        import concourse  # noqa: F401
        return
    except ImportError:
        pass
    for p in ("/opt/trn_rl_repo", os.path.expanduser("~/.axon_site/_ro/trn_rl_repo")):
        if os.path.isdir(p):
            sys.path.insert(0, p)
            try:
                import concourse  # noqa: F401
                return
            except ImportError:
                sys.path.remove(p)
    raise ImportError("concourse (bass) not importable")


_ensure_concourse()

import concourse.bass as bass  # noqa: E402
import concourse.mybir as mybir  # noqa: E402
import concourse.tile as tile  # noqa: E402
from concourse import bacc  # noqa: E402
from concourse.bass_utils import run_bass_kernel_spmd  # noqa: E402

F32 = mybir.dt.float32
BF16 = mybir.dt.bfloat16
FP8 = mybir.dt.float8e4

C = 128          # channels / hidden dim
B = 4            # batch
N = 4096         # tokens per batch (64*64)
NQ = 2048        # query tokens per core (half batch)
N_CORES = 8
NJT = N // 128   # 32 key chunks of 128
N_IB = NQ // 512  # 4 query blocks of 512
NG = NJT // 2    # 16 key-chunk pairs per query block
LAG = 3          # PV trails exp by this many groups
SCALE = float(1.0 / np.sqrt(C))
UNROLL = 8       # bodies per hardware-loop iteration (amortizes the
                 # all-engine barrier For_i inserts between iterations)


def build_nc(repeats=1):
    nc = bacc.Bacc("TRN2", target_bir_lowering=False, debug=False,
                   num_devices=N_CORES)

    x1bf = nc.declare_dram_parameter("x1bf", [C, NQ], BF16, isOutput=False)
    x2f = nc.declare_dram_parameter("x2f", [C, N], BF16, isOutput=False)
    wcT = nc.declare_dram_parameter("wcT", [C, C], BF16, isOutput=False)
    wvT = nc.declare_dram_parameter("wvT", [C, C], BF16, isOutput=False)
    bcv = nc.declare_dram_parameter("bcv", [C, 2], F32, isOutput=False)
    out = nc.declare_dram_parameter("out", [C, NQ], F32, isOutput=True)

    with tile.TileContext(nc) as tc:
        with (
            tc.tile_pool(name="const", bufs=1) as cpool,
            tc.tile_pool(name="persist", bufs=1) as ppool,
            tc.tile_pool(name="work", bufs=2) as wpool,
            tc.tile_pool(name="ps_s", bufs=2, space="PSUM") as ps_s,
            tc.tile_pool(name="ps_od", bufs=2, space="PSUM") as ps_od,
            tc.tile_pool(name="ps_aux", bufs=2, space="PSUM") as ps_aux,
        ):
            pools = (cpool, ppool, wpool, ps_s, ps_od, ps_aux)
            if repeats == 1:
                _build_body(nc, pools, x1bf, x2f, wcT, wvT, bcv, out)
            else:
                hints = (mybir.EngineType.PE, mybir.EngineType.Activation,
                         mybir.EngineType.DVE, mybir.EngineType.SP,
                         mybir.EngineType.Pool)
                u = UNROLL if repeats % UNROLL == 0 else 1
                with tc.For_i(0, repeats // u, 1, hint_engines=hints):
                    for _ in range(u):
                        _build_body(nc, pools, x1bf, x2f, wcT, wvT,
                                    bcv, out)

    nc.compile()
    return nc


def _build_body(nc, pools, x1bf, x2f, wcT, wvT, bcv, out):
    (cpool, ppool, wpool, ps_s, ps_od, ps_aux) = pools

    # ---- constants -----------------------------------------------------
    wc_b = cpool.tile([C, C], BF16, tag="wcb", name="wcb")
    nc.sync.dma_start(wc_b[:], wcT[:])
    bias_t = cpool.tile([C, 2], F32, tag="bias", name="bias_t")
    b_c = bias_t[:, 0:1]
    b_v = bias_t[:, 1:2]
    # DoubleRow all-ones weights with M=128: the den matmul then writes
    # den broadcast across all 128 partitions, so the epilogue needs no
    # separate PE broadcast of 1/den.
    ones_dr = cpool.tile([C, 2, 128], FP8, tag="ones_dr", name="ones_dr")
    nc.vector.memset(ones_dr[:], 1.0)

    # ---- input DMAs, ordered by need time ------------------------------
    # x1bf(0) + wc gate the first qt projection; x2 chunks gate the S
    # stream; x1h (residual) is needed only at each block epilogue.
    x1b = [None] * N_IB
    x2b = [None] * 8
    x1c = [None] * N_IB

    def dma_x1b(ci):
        xb = ppool.tile([C, 512], BF16, tag=f"x1b{ci}", name=f"x1b{ci}")
        nc.sync.dma_start(xb[:], x1bf[:, ci * 512:(ci + 1) * 512])
        x1b[ci] = xb

    def dma_x2b(ci):
        xb = ppool.tile([C, 512], BF16, tag=f"x2b{ci}", name=f"x2b{ci}")
        nc.sync.dma_start(xb[:], x2f[:, ci * 512:(ci + 1) * 512])
        x2b[ci] = xb

    dma_x1b(0)
    nc.sync.dma_start(bias_t[:], bcv[:])
    wv_b = cpool.tile([C, C], BF16, tag="wvb", name="wvb")
    nc.sync.dma_start(wv_b[:], wvT[:])
    for ci in range(8):
        dma_x2b(ci)
    dma_x1b(1)
    dma_x1b(2)
    dma_x1b(3)

    # ---- helpers -------------------------------------------------------
    qt = [None] * N_IB

    def emit_qt(ib):
        q_ps = ps_aux.tile([C, 512], F32, tag="aux", name=f"qps{ib}")
        nc.tensor.matmul(q_ps[:], wc_b[:], x1b[ib][:], start=True, stop=True)
        q = ppool.tile([C, 512], BF16, tag=f"qt{ib}", name=f"qt{ib}")
        nc.vector.tensor_scalar_add(q[:], q_ps[:], b_c[:])
        qt[ib] = q

    vquad = [None] * 8

    def emit_vq(ci):
        # v rows for key chunks 4ci..4ci+3: four projections into one PSUM
        # bank, one DVE copy out to an fp8 [128, 4, C] tile whose [.,2h:2h+2,.]
        # slices are the DoubleRow lhsT pairs.
        v_ps = ps_aux.tile([128, 512], F32, tag="aux", name=f"vps{ci}")
        for h in range(4):
            jt = 4 * ci + h
            lhs = x2b[jt // 4][:, (jt % 4) * 128:(jt % 4) * 128 + 128]
            nc.tensor.matmul(v_ps[:, h * 128:(h + 1) * 128], lhs, wv_b[:],
                             start=True, stop=True)
        vq = ppool.tile([128, 4, C], FP8, tag=f"vq{ci}", name=f"vq{ci}")
        nc.vector.tensor_copy(vq.rearrange("p f c -> p (f c)"), v_ps[:])
        vquad[ci] = vq

    def emit_xc_bias(ci):
        # residual base: x1 + bv (bf16 x1 is within tolerance; bv exact
        # after softmax normalization)
        xc = ppool.tile([C, 512], F32, tag=f"x1c{ci}", name=f"x1c{ci}")
        nc.vector.tensor_scalar_add(xc[:], x1b[ci][:], b_v[:])
        x1c[ci] = xc

    def emit_s(ib, g):
        jt0, jt1 = 2 * g, 2 * g + 1
        s_ps = ps_s.tile([128, 1024], F32, tag="s", name=f"sps{ib}_{g}")
        k0 = x2b[jt0 // 4][:, (jt0 % 4) * 128:(jt0 % 4) * 128 + 128]
        k1 = x2b[jt1 // 4][:, (jt1 % 4) * 128:(jt1 % 4) * 128 + 128]
        nc.tensor.matmul(s_ps[:, 0:512], k0, qt[ib][:], start=True, stop=True)
        nc.tensor.matmul(s_ps[:, 512:1024], k1, qt[ib][:],
                         start=True, stop=True)
        e = wpool.tile([128, 1024], FP8, tag="e", bufs=8, name=f"e{ib}_{g}")
        nc.scalar.activation(e[:], s_ps[:], mybir.ActivationFunctionType.Exp,
                             bias=0.0, scale=SCALE)
        return e

    def emit_pv(g, e, o_ps, den_ps):
        first = g == 0
        last = g == NG - 1
        epair = e.rearrange("p (two n) -> p two n", two=2)
        vp = vquad[g // 2][:, 2 * (g % 2):2 * (g % 2) + 2, :]
        nc.tensor.matmul(o_ps[:], vp, epair, start=first, stop=last,
                         perf_mode=mybir.MatmulPerfMode.DoubleRow)
        nc.tensor.matmul(den_ps[:], ones_dr[:], epair,
                         start=first, stop=last,
                         perf_mode=mybir.MatmulPerfMode.DoubleRow)

    def emit_epilogue(ib, o_ps, den_ps):
        # den_ps is already broadcast across partitions (ones_dr M=128):
        # 1/den -> SBUF, multiply into o, add residual, store.
        recip = wpool.tile([C, 512], F32, tag="recip", bufs=2,
                           name=f"recip{ib}")
        nc.vector.reciprocal(recip[:], den_ps[:])
        ob = wpool.tile([C, 512], F32, tag="ob", bufs=2, name=f"ob{ib}")
        nc.vector.tensor_mul(ob[:], o_ps[:], recip[:])
        nc.vector.tensor_add(ob[:], ob[:], x1c[ib][:])
        nc.sync.dma_start(out[:, ib * 512:(ib + 1) * 512], ob[:])

    # ---- main attention stream -----------------------------------------
    # Block ib: 16 groups of (2 S-matmuls -> exp), with PV/den trailing by
    # LAG groups so ScalarE exp latency hides behind PE work. V projections
    # are emitted during block 0 just ahead of first use; qt(ib+1) is
    # emitted early inside block ib.
    emit_qt(0)
    for ib in range(N_IB):
        o_ps = ps_od.tile([C, 512], F32, tag="od", name=f"ops{ib}")
        den_ps = ps_od.tile([C, 512], F32, tag="od", name=f"den{ib}")
        e_q = []
        for g in range(NG):
            if ib == 0 and g < 4:
                # front-load the first quads (PV of pair p starts at g=p+LAG)
                emit_vq(g)
            elif ib == 0 and g % 2 == 0 and g // 2 + 2 < 8:
                # then one quad every other group to keep PE under the exp
                # cadence (quad ci covers pairs 2ci, 2ci+1, used at g=2ci+LAG)
                emit_vq(g // 2 + 2)
            e_q.append(emit_s(ib, g))
            if ib == 0 and g == 4:
                emit_xc_bias(0)
            if g == 5 and ib + 1 < N_IB:
                emit_qt(ib + 1)
            if g == 7 and ib + 1 < N_IB:
                emit_xc_bias(ib + 1)
            if g >= LAG:
                emit_pv(g - LAG, e_q[g - LAG], o_ps, den_ps)
        # flush trailing PV groups, then normalize + residual + store
        for g in range(NG - LAG, NG):
            emit_pv(g, e_q[g], o_ps, den_ps)
        emit_epilogue(ib, o_ps, den_ps)


_NC_CACHE = None


def _get_nc():
    global _NC_CACHE
    if _NC_CACHE is None:
        _NC_CACHE = build_nc()
    return _NC_CACHE


def make_in_maps(x1, x2, wq, bq, wk, bk, wv, bv):
    x1 = np.asarray(x1, np.float32)
    x2 = np.asarray(x2, np.float32)
    t1 = np.ascontiguousarray(x1.reshape(B, C, N))
    t2 = np.ascontiguousarray(x2.reshape(B, C, N))
    import ml_dtypes
    bf = ml_dtypes.bfloat16
    wq = np.asarray(wq, np.float64)
    wk = np.asarray(wk, np.float64)
    # fused q/k projection: S^T = x2^T (Wc x1 + bc), bk cancels in softmax
    wc = wq.T @ wk                      # lhsT layout [c_in(x1), c_mid]
    bc = wk.T @ np.asarray(bq, np.float64)
    shared = {
        "wcT": np.ascontiguousarray(wc.astype(np.float32).astype(bf)),
        "wvT": np.ascontiguousarray(
            np.asarray(wv, np.float32).T.astype(bf)),
        "bcv": np.ascontiguousarray(np.stack(
            [bc.astype(np.float32), np.asarray(bv, np.float32)], axis=1)),
    }
    in_maps = []
    for core in range(N_CORES):
        b, h = core // 2, core % 2
        in_maps.append({
            "x1bf": np.ascontiguousarray(
                t1[b][:, h * NQ:(h + 1) * NQ]).astype(bf),
            "x2f": t2[b].astype(bf),
            **shared,
        })
    return in_maps


def assemble_out(results):
    out = np.empty((B, C, N), np.float32)
    for core in range(N_CORES):
        b, h = core // 2, core % 2
        out[b][:, h * NQ:(h + 1) * NQ] = results[core]["out"]
    return out.reshape(B, C, 64, 64)


def kernel(x1, x2, wq, bq, wk, bk, wv, bv):
    nc = _get_nc()
    in_maps = make_in_maps(x1, x2, wq, bq, wk, bk, wv, bv)
    res = run_bass_kernel_spmd(nc, in_maps, list(range(N_CORES)))
    return assemble_out(res.results)
